# revision 11
# baseline (speedup 1.0000x reference)
"""Trainium2 Bass kernel for nn_AdaptiveLayoutEncoder (flow-encoder + 3x GAT).

Strategy: data-parallel over batch (64 graphs / core on 8 cores).
GAT segment ops are reformulated densely via a per-graph edge-count matrix
C[dst,src] built on-device with one-hot matmuls (attention coefficients only
depend on (src,dst) node ids, so duplicate edges multiply the exp weight).
Softmax max-subtraction is dropped (args bounded ~|2|; the ratio is exact).
LayerNorm affine params + all linear biases are folded on the host into the
weight matrices / small bias tiles applied during PSUM->SBUF evacuations.
All attention/e matrices are built in transposed (source-on-partition)
orientation so softmax denominators come from gpsimd partition_all_reduce
and aggregation matmuls need no transposes.
"""

import sys

sys.path.insert(0, "/opt/trn_rl_repo")

import numpy as np
import ml_dtypes

import concourse.bass as bass
import concourse.tile as tile
from concourse import bacc, mybir, bass_isa
from concourse.bass_utils import run_bass_kernel_spmd

BF = mybir.dt.bfloat16
F32 = mybir.dt.float32
I32 = mybir.dt.int32

HID, HEADS, HC, LAYERS = 128, 4, 32, 3
B, N, E, ATTR = 512, 100, 1024, 7
NCORES = 8
GPC = B // NCORES  # graphs per core
LN_EPS = 1e-5
NEG = 0.2
ECH = E // 128  # edge chunks of 128

bf16 = ml_dtypes.bfloat16
AL = mybir.AluOpType
AF = mybir.ActivationFunctionType


def _ap(t, *dims):
    """AP on tile t: partition pair from the tile, explicit free dims."""
    a = t[:]
    return bass.AP(tensor=a.tensor, offset=a.offset, ap=[a.ap[0], *dims])


def build_graph(n_graphs=GPC):
    nc = bacc.Bacc(None, target_bir_lowering=False)

    deptT = nc.declare_dram_parameter("deptT", [n_graphs, ATTR + 1, N], BF, isOutput=False)
    flowT = nc.declare_dram_parameter("flowT", [n_graphs, N + 1, N], BF, isOutput=False)
    edges = nc.declare_dram_parameter("edges", [n_graphs, 2, E], I32, isOutput=False)
    out_e = nc.declare_dram_parameter("out", [n_graphs, N, HID], F32, isOutput=True)

    pspecs = [
        ("dwA", [ATTR + 1, HID], BF), ("fwA", [N + 1, HID], BF),
        ("wq2", [HID, HID], BF), ("wk2", [HID, HID], BF),
        ("wv2", [HID, HID], BF), ("wo2", [HID, HID], BF),
        ("fuswT", [HID, HID], BF), ("fuswB", [HID, HID], BF),
        ("gWf", [LAYERS, HID, HID], BF), ("A_all", [LAYERS, HID, 2 * HEADS], BF),
        ("woc", [HC, HEADS, HID], BF), ("gWc", [LAYERS, HC, HEADS, HID], BF),
        ("bq2r", [HC, HEADS], F32), ("bk2r", [HC, HEADS], F32),
        ("gbr", [2, HC, HEADS], F32),
        ("bq2c", [HID, 1], F32), ("bk2c", [HID, 1], F32),
        ("bo2c", [HID, 1], F32), ("c1c", [HID, 1], F32),
        ("gbc", [HID, LAYERS], F32),
        ("bv2f", [N, HID], F32), ("fusb2f", [N, HID], F32),
        ("c1f", [N, HID], F32), ("gb3f", [N, HID], F32),
        ("identb", [128, 128], BF), ("eyef", [N, N], F32),
        ("iotab", [128, N], BF), ("ones1", [1, 128], BF),
        ("epsc", [128, 1], F32),
    ]
    drams = {nm: nc.declare_dram_parameter(nm, sh, dt, isOutput=False) for nm, sh, dt in pspecs}

    with tile.TileContext(nc) as tc:
        with (
            tc.tile_pool(name="pers", bufs=1) as pers,
            tc.tile_pool(name="work", bufs=3) as work,
            tc.tile_pool(name="wide", bufs=2) as wide,
            tc.tile_pool(name="ps_mm", bufs=4, space="PSUM") as ps_mm,
            tc.tile_pool(name="ps_w", bufs=2, space="PSUM") as ps_w,
            tc.tile_pool(name="ps_c", bufs=2, space="PSUM") as ps_c,
        ):
            P = {}
            PER_LAYER = {"gWf": [HID, HID], "gWc": [HC, HEADS, HID],
                         "A_all": [HID, 2 * HEADS], "gbr": [HC, HEADS]}
            for nm, sh, dt in pspecs:
                if nm in PER_LAYER:
                    P[nm] = []
                    for l in range(sh[0]):
                        t = pers.tile(PER_LAYER[nm], dt, tag=f"p_{nm}{l}",
                                      name=f"p_{nm}{l}")
                        nc.sync.dma_start(t[:], drams[nm][l])
                        P[nm].append(t)
                else:
                    t = pers.tile(list(sh), dt, tag=f"p_{nm}", name=f"p_{nm}")
                    nc.sync.dma_start(t[:], drams[nm][:])
                    P[nm] = t

            _mmc = [0]

            def mmtile():
                _mmc[0] += 1
                return ps_mm.tile([128, 128], F32, tag="mm", name=f"mm{_mmc[0]}")

            def cmtile():
                _mmc[0] += 1
                return ps_mm.tile([HC, HEADS, N], F32, tag="mm", name=f"cm{_mmc[0]}")

            def ln_hat(pre_ps, bias_full, outname):
                """relu(pre+bias) -> LN (no affine) -> bf16 node-major (N,HID)."""
                rl = work.tile([N, HID], F32, tag="ln_rl")
                if bias_full is not None:
                    nc.vector.scalar_tensor_tensor(
                        rl[:], pre_ps[:N, :HID], 1.0, bias_full[:],
                        op0=AL.mult, op1=AL.add)
                    nc.scalar.activation(rl[:], rl[:], AF.Relu)
                else:
                    nc.scalar.activation(rl[:], pre_ps[:N, :HID], AF.Relu)
                st = work.tile([N, 6], F32, tag="ln_st")
                mv = work.tile([N, 2], F32, tag="ln_mv")
                nc.vector.bn_stats(out=st[:], in_=rl[:])
                nc.vector.bn_aggr(out=mv[:], in_=st[:])
                sd = work.tile([N, 1], F32, tag="ln_sd")
                nc.scalar.activation(sd[:], mv[:, 1:2], AF.Sqrt,
                                     bias=P["epsc"][:N, :], scale=1.0)
                nc.vector.reciprocal(sd[:], sd[:])
                hat = work.tile([N, HID], BF, tag=f"{outname}_hat")
                nc.vector.tensor_scalar(hat[:], rl[:], mv[:, 0:1], sd[:],
                                        op0=AL.subtract, op1=AL.mult)
                return hat

            def transpose_nm(hat, outname):
                """(N,HID) bf16 node-major -> (HID,N) bf16 hid-major."""
                ps = mmtile()
                nc.tensor.matmul(ps[:HID, :N], hat[:], P["identb"][:N, :N],
                                 start=True, stop=True)
                ht = work.tile([HID, N], BF, tag=f"{outname}_T")
                nc.vector.tensor_copy(ht[:], ps[:HID, :N])
                return ht

            for g in range(n_graphs):
                # ================= C matrix =================
                ed = work.tile([128, 2, ECH], I32, tag="ed")
                nc.sync.dma_start(ed[:], edges[g].rearrange("t (c p) -> p t c", p=128))
                edb = work.tile([128, 2, ECH], BF, tag="edb")
                nc.vector.tensor_copy(edb[:], ed[:])
                oh = work.tile([128, 2, ECH, N], BF, tag="oh")
                nc.vector.tensor_tensor(
                    oh[:],
                    _ap(P["iotab"], [0, 2], [0, ECH], [1, N]),
                    _ap(edb, [ECH, 2], [1, ECH], [0, N]),
                    op=AL.is_equal)
                psC = ps_c.tile([N, N], F32, tag="c")
                for c in range(ECH):
                    nc.tensor.matmul(psC[:], oh[:, 1, c, :], oh[:, 0, c, :],
                                     start=(c == 0), stop=(c == ECH - 1))
                Craw = work.tile([N, N], BF, tag="Craw")
                nc.vector.tensor_copy(Craw[:], psC[:])
                psCT = ps_c.tile([N, N], F32, tag="c")
                nc.tensor.matmul(psCT[:], Craw[:], P["identb"][:N, :N],
                                 start=True, stop=True)
                CT = work.tile([N, N], BF, tag="CT")
                nc.vector.tensor_tensor(CT[:], psCT[:], P["eyef"][:], op=AL.add)

                # ================= flow encoder =================
                dT_aug = work.tile([ATTR + 1, N], BF, tag="dT_aug")
                nc.sync.dma_start(dT_aug[:], deptT[g])
                ps_d = mmtile()
                nc.tensor.matmul(ps_d[:N, :HID], dT_aug[:], P["dwA"][:],
                                 start=True, stop=True)
                dhat = ln_hat(ps_d, None, "d")
                dhatT = transpose_nm(dhat, "d")

                fT_aug = work.tile([N + 1, N], BF, tag="fT_aug")
                nc.sync.dma_start(fT_aug[:], flowT[g])
                ps_f = mmtile()
                nc.tensor.matmul(ps_f[:N, :HID], fT_aug[:], P["fwA"][:],
                                 start=True, stop=True)
                fhat = ln_hat(ps_f, None, "f")
                fhatT = transpose_nm(fhat, "f")

                ps_qc = cmtile()
                for h in range(HEADS):
                    nc.tensor.matmul(ps_qc[:, h, :], P["wq2"][:, 32 * h:32 * h + 32],
                                     dhatT[:], start=True, stop=True)
                qTc = work.tile([HC, HEADS, N], BF, tag="qTc")
                nc.vector.tensor_tensor(qTc[:], ps_qc[:],
                                        _ap(P["bq2r"], [1, HEADS], [0, N]), op=AL.add)
                ps_kc = cmtile()
                for h in range(HEADS):
                    nc.tensor.matmul(ps_kc[:, h, :], P["wk2"][:, 32 * h:32 * h + 32],
                                     fhatT[:], start=True, stop=True)
                kTc = work.tile([HC, HEADS, N], BF, tag="kTc")
                nc.vector.tensor_tensor(kTc[:], ps_kc[:],
                                        _ap(P["bk2r"], [1, HEADS], [0, N]), op=AL.add)
                ps_v = mmtile()
                nc.tensor.matmul(ps_v[:N, :HID], fhatT[:], P["wv2"][:],
                                 start=True, stop=True)
                vb = work.tile([N, HID], BF, tag="vb")
                nc.vector.scalar_tensor_tensor(vb[:], ps_v[:N, :HID], 1.0,
                                               P["bv2f"][:], op0=AL.mult, op1=AL.add)

                # scores transposed: scT[kn, (h,qn)]
                ps_sc = ps_w.tile([N, HEADS, N], F32, tag="w")
                for h in range(HEADS):
                    nc.tensor.matmul(ps_sc[:, h, :], kTc[:, h, :], qTc[:, h, :],
                                     start=True, stop=True)
                ee = wide.tile([N, HEADS, N], F32, tag="ee")
                nc.scalar.activation(ee[:], ps_sc[:], AF.Exp)
                den = wide.tile([N, HEADS, N], F32, tag="den")
                nc.gpsimd.partition_all_reduce(den[:], ee[:], channels=N,
                                               reduce_op=bass_isa.ReduceOp.add)
                nc.vector.reciprocal(den[:], den[:])
                aaT = wide.tile([N, HEADS, N], BF, tag="aaT")
                nc.vector.tensor_tensor(aaT[:], ee[:], den[:], op=AL.mult)

                ps_atc = cmtile()
                for h in range(HEADS):
                    nc.tensor.matmul(ps_atc[:, h, :],
                                     vb[:, 32 * h:32 * h + 32], aaT[:, h, :],
                                     start=True, stop=True)
                attc = work.tile([HC, HEADS, N], BF, tag="attc")
                nc.vector.tensor_copy(attc[:], ps_atc[:])
                ps_a2 = mmtile()
                for h in range(HEADS):
                    nc.tensor.matmul(ps_a2[:HID, :N], P["woc"][:, h, :],
                                     attc[:, h, :],
                                     start=(h == 0), stop=(h == HEADS - 1))
                a2T = work.tile([HID, N], BF, tag="a2T")
                nc.vector.tensor_scalar_add(a2T[:], ps_a2[:HID, :N], P["bo2c"][:])

                ps_fu = mmtile()
                nc.tensor.matmul(ps_fu[:N, :HID], dhatT[:], P["fuswT"][:],
                                 start=True, stop=False)
                nc.tensor.matmul(ps_fu[:N, :HID], a2T[:], P["fuswB"][:],
                                 start=False, stop=True)
                hhat = ln_hat(ps_fu, P["fusb2f"], "h0")
                hT = transpose_nm(hhat, "h0")

                # ================= GAT layers =================
                hTc = None  # head-chunked (HC, HEADS, N) for layers 1,2
                for l in range(LAYERS):
                    ps_x3T = mmtile()
                    if l == 0:
                        nc.tensor.matmul(ps_x3T[:HID, :N], P["gWf"][l], hT[:],
                                         start=True, stop=True)
                    else:
                        for h in range(HEADS):
                            nc.tensor.matmul(ps_x3T[:HID, :N], P["gWc"][l][:, h, :],
                                             hTc[:, h, :],
                                             start=(h == 0), stop=(h == HEADS - 1))
                    x3T = work.tile([HID, N], BF, tag="x3T")
                    if l == 0:
                        nc.vector.tensor_scalar_add(x3T[:], ps_x3T[:HID, :N],
                                                    P["c1c"][:])
                    else:
                        nc.vector.tensor_copy(x3T[:], ps_x3T[:HID, :N])
                    ps_x3 = mmtile()
                    if l == 0:
                        nc.tensor.matmul(ps_x3[:N, :HID], hT[:], P["gWf"][l],
                                         start=True, stop=True)
                    else:
                        for h in range(HEADS):
                            nc.tensor.matmul(ps_x3[:N, :HID], hTc[:, h, :],
                                             P["gWc"][l][:, h, :],
                                             start=(h == 0), stop=(h == HEADS - 1))
                    x3 = work.tile([N, HID], BF, tag="x3")
                    if l == 0:
                        nc.vector.scalar_tensor_tensor(
                            x3[:], ps_x3[:N, :HID], 1.0, P["c1f"][:],
                            op0=AL.mult, op1=AL.add)
                    else:
                        nc.vector.tensor_copy(x3[:], ps_x3[:N, :HID])

                    ps_al = mmtile()
                    nc.tensor.matmul(ps_al[:HEADS, :N], P["A_all"][l][:, :HEADS],
                                     x3T[:], start=True, stop=True)
                    alb = work.tile([HEADS, N], BF, tag="alb")
                    nc.vector.tensor_copy(alb[:], ps_al[:HEADS, :N])
                    ps_asT = mmtile()
                    nc.tensor.matmul(ps_asT[:N, :HEADS], alb[:],
                                     P["identb"][:HEADS, :HEADS],
                                     start=True, stop=True)
                    asT = work.tile([N, HEADS], F32, tag="asT")
                    nc.vector.tensor_copy(asT[:], ps_asT[:N, :HEADS])
                    ps_adc_t = cmtile()
                    ps_adc = ps_adc_t[:1]
                    for h in range(HEADS):
                        nc.tensor.matmul(ps_adc[:, h, :],
                                         P["A_all"][l][:, HEADS + h:HEADS + h + 1],
                                         x3T[:], start=True, stop=True)
                    ad4 = work.tile([1, HEADS, N], BF, tag="ad4")
                    nc.vector.tensor_copy(ad4[:], ps_adc[:])

                    # e transposed: e[sn,(h,dn)] = leaky(as[sn] + ad[dn])
                    ps_e = ps_w.tile([N, HEADS, N], F32, tag="w")
                    for h in range(HEADS):
                        nc.tensor.matmul(ps_e[:, h, :], P["ones1"][:, :N],
                                         ad4[:, h, :], start=True, stop=True)
                    epre = wide.tile([N, HEADS, N], F32, tag="epre")
                    nc.vector.tensor_tensor(
                        epre[:], ps_e[:], _ap(asT, [1, HEADS], [0, N]), op=AL.add)
                    el = wide.tile([N, HEADS, N], F32, tag="el")
                    nc.vector.scalar_tensor_tensor(el[:], epre[:], NEG, epre[:],
                                                   op0=AL.mult, op1=AL.max)
                    eeg = wide.tile([N, HEADS, N], F32, tag="eeg")
                    nc.scalar.activation(eeg[:], el[:], AF.Exp)
                    eec = wide.tile([N, HEADS, N], F32, tag="eec")
                    nc.vector.tensor_tensor(
                        eec[:], eeg[:], _ap(CT, [0, HEADS], [1, N]), op=AL.mult)
                    deng = wide.tile([N, HEADS, N], F32, tag="deng")
                    nc.gpsimd.partition_all_reduce(deng[:], eec[:], channels=N,
                                                   reduce_op=bass_isa.ReduceOp.add)
                    nc.vector.reciprocal(deng[:], deng[:])
                    aaTg = wide.tile([N, HEADS, N], BF, tag="aaTg")
                    nc.vector.tensor_tensor(aaTg[:], eec[:], deng[:], op=AL.mult)

                    if l < LAYERS - 1:
                        ps_hgc = cmtile()
                        for h in range(HEADS):
                            nc.tensor.matmul(ps_hgc[:, h, :],
                                             x3[:, 32 * h:32 * h + 32],
                                             aaTg[:, h, :],
                                             start=True, stop=True)
                        hb = work.tile([HC, HEADS, N], F32, tag="hb")
                        nc.vector.tensor_tensor(hb[:], ps_hgc[:],
                                                _ap(P["gbr"][l], [1, HEADS], [0, N]),
                                                op=AL.add)
                        hTc = work.tile([HC, HEADS, N], BF, tag=f"hTc{l}")
                        nc.scalar.activation(hTc[:], hb[:], AF.Relu)
                    else:
                        ps_h3 = mmtile()
                        for h in range(HEADS):
                            nc.tensor.matmul(ps_h3[:N, 32 * h:32 * h + 32],
                                             aaTg[:, h, :],
                                             x3[:, 32 * h:32 * h + 32],
                                             start=True, stop=True)
                        outf = work.tile([N, HID], F32, tag="outf")
                        nc.vector.scalar_tensor_tensor(
                            outf[:], ps_h3[:N, :HID], 1.0, P["gb3f"][:],
                            op0=AL.mult, op1=AL.add)
                        nc.scalar.activation(outf[:], outf[:], AF.Relu)
                        nc.sync.dma_start(out_e[g], outf[:])
    return nc


# ---------------- host side ----------------

def _prep(dept_attrs, flow_matrix, params):
    p = {k: np.asarray(v, np.float32) for k, v in params.items()}
    sq = 1.0 / np.sqrt(HC)
    dwA = np.concatenate([p['dw'], p['db'][None]], 0)
    fwA = np.concatenate([p['fw'], p['fb'][None]], 0)
    wq2 = (p['dg'][:, None] * p['wq']) * sq
    bq2 = (p['dbb'] @ p['wq'] + p['bq']) * sq
    wk2 = p['fg'][:, None] * p['wk']
    bk2 = p['fbb'] @ p['wk'] + p['bk']
    wv2 = p['fg'][:, None] * p['wv']
    bv2 = p['fbb'] @ p['wv'] + p['bv']
    fuswT = p['dg'][:, None] * p['fusw'][:HID]
    fuswB = p['fusw'][HID:]
    fusb2 = p['dbb'] @ p['fusw'][:HID] + p['fusb']
    gW0 = p['gW'][0]
    gWf = np.stack([p['fusg'][:, None] * gW0, p['gW'][1], p['gW'][2]], 0)
    c1 = p['fusbb'] @ gW0
    A_all = np.zeros((LAYERS, HID, 2 * HEADS), np.float32)
    for l in range(LAYERS):
        for h in range(HEADS):
            A_all[l, 32 * h:32 * h + 32, h] = p['g_asrc'][l, h]
            A_all[l, 32 * h:32 * h + 32, HEADS + h] = p['g_adst'][l, h]
    gbc = np.stack([p['g_b'][0], p['g_b'][1], p['g_b'][2]], 1)
    woc = p['wo'].reshape(HEADS, HC, HID).transpose(1, 0, 2)
    gWc = gWf.reshape(LAYERS, HEADS, HC, HID).transpose(0, 2, 1, 3)
    bq2r = bq2.reshape(HEADS, HC).T
    bk2r = bk2.reshape(HEADS, HC).T
    gbr = np.stack([p['g_b'][0].reshape(HEADS, HC).T,
                    p['g_b'][1].reshape(HEADS, HC).T], 0)

    nb = dept_attrs.shape[0]
    dept = np.asarray(dept_attrs, np.float32)
    deptT = np.concatenate(
        [dept.transpose(0, 2, 1), np.ones((nb, 1, N), np.float32)], 1)
    flow = np.asarray(flow_matrix, np.float32)
    flowT = np.concatenate(
        [flow.transpose(0, 2, 1), np.ones((nb, 1, N), np.float32)], 1)

    iota = np.broadcast_to(np.arange(N, dtype=np.float32), (128, N))
    const = dict(
        dwA=dwA.astype(bf16), fwA=fwA.astype(bf16), wq2=wq2.astype(bf16),
        wk2=wk2.astype(bf16), wv2=wv2.astype(bf16), wo2=p['wo'].astype(bf16),
        fuswT=fuswT.astype(bf16), fuswB=fuswB.astype(bf16),
        gWf=gWf.astype(bf16), A_all=A_all.astype(bf16),
        woc=woc.astype(bf16), gWc=gWc.astype(bf16),
        bq2r=np.ascontiguousarray(bq2r, np.float32),
        bk2r=np.ascontiguousarray(bk2r, np.float32),
        gbr=np.ascontiguousarray(gbr, np.float32),
        bq2c=np.ascontiguousarray(bq2[:, None], np.float32),
        bk2c=np.ascontiguousarray(bk2[:, None], np.float32),
        bo2c=np.ascontiguousarray(p['bo'][:, None], np.float32),
        c1c=np.ascontiguousarray(c1[:, None], np.float32),
        gbc=np.ascontiguousarray(gbc, np.float32),
        bv2f=np.ascontiguousarray(np.broadcast_to(bv2, (N, HID)), np.float32),
        fusb2f=np.ascontiguousarray(np.broadcast_to(fusb2, (N, HID)), np.float32),
        c1f=np.ascontiguousarray(np.broadcast_to(c1, (N, HID)), np.float32),
        gb3f=np.ascontiguousarray(np.broadcast_to(p['g_b'][2], (N, HID)), np.float32),
        identb=np.eye(128, dtype=bf16), eyef=np.eye(N, dtype=np.float32),
        iotab=np.ascontiguousarray(iota).astype(bf16),
        ones1=np.ones((1, 128), bf16),
        epsc=np.full((128, 1), LN_EPS, np.float32),
    )
    return deptT.astype(bf16), flowT.astype(bf16), const


_CACHE = {}


def kernel(dept_attrs, flow_matrix, edge_index, edge_weight, node_mask, params):
    deptT, flowT, const = _prep(dept_attrs, flow_matrix, params)
    edges = np.ascontiguousarray(np.asarray(edge_index, np.int32))

    if "nc" not in _CACHE:
        nc = build_graph(GPC)
        nc.compile()
        _CACHE["nc"] = nc
    nc = _CACHE["nc"]

    in_maps = []
    for c in range(NCORES):
        sl = slice(c * GPC, (c + 1) * GPC)
        m = dict(deptT=np.ascontiguousarray(deptT[sl]),
                 flowT=np.ascontiguousarray(flowT[sl]),
                 edges=np.ascontiguousarray(edges[sl]))
        m.update(const)
        in_maps.append(m)

    res = run_bass_kernel_spmd(nc, in_maps, core_ids=list(range(NCORES)))
    _CACHE["last_res"] = res
    out = np.concatenate([res.results[c]["out"] for c in range(NCORES)], 0)
    return out.astype(np.float32)
